# revision 3
# baseline (speedup 1.0000x reference)
"""Trainium2 Bass kernel for a dense pre-norm transformer block.

Reference semantics (per batch b, seq len T=100, d_model D=384, heads H=6):
  h   = LN(x) * g1 + beta1
  q,k,v = per-head projections of h;  wei = softmax(mask(q k^T * sqrt(64)))
  x2  = x + (wei v) Wp + bp
  out = x2 + relu(LN(x2)*g2+beta2 @ W1 + b1) @ W2 + b2

Distribution: data-parallel over the batch dim across 8 NeuronCores
(64 batches each); weights are replicated. No collectives.

Host-side folds: g/beta of each LN are folded into the following matmul
weights/biases; the sqrt(head_size) score scale is folded into Wq.
All big matmuls run as float32r (reduced-precision fp32, full PE rate);
residual adds stay fp32.
"""

import numpy as np
from contextlib import ExitStack

B, T, D = 512, 100, 384
H, HS = 6, 64
DH = 4 * D                      # FFN hidden 1536
N_CORES = 8
BC = B // N_CORES               # batches per core
EPS = 1e-5
MASK_VAL = -1e9
NB = 5                          # batches per inner group
NTOK_MAX = NB * T               # 500
KC_D = D // 128                 # 3 contraction chunks over D
KC_H = DH // 128                # 12 contraction chunks over DH

_NC_CACHE = {}


def _build_nc(use_bv, use_bp, use_b2):
    import concourse.bass as bass
    import concourse.tile as tile
    from concourse import bacc, mybir

    f32 = mybir.dt.float32
    f32r = mybir.dt.float32r
    AF = mybir.ActivationFunctionType
    ts = bass.ts

    nc = bacc.Bacc("TRN2", target_bir_lowering=False, debug=False,
                   enable_asserts=True, num_devices=N_CORES)

    x_d = nc.dram_tensor("x", [BC, T, D], f32, kind="ExternalInput").ap()
    wq_d = nc.dram_tensor("wq_l", [128, KC_D, D], f32r, kind="ExternalInput").ap()
    wk_d = nc.dram_tensor("wk_l", [128, KC_D, D], f32r, kind="ExternalInput").ap()
    wv_d = nc.dram_tensor("wv_l", [128, KC_D, D], f32r, kind="ExternalInput").ap()
    wp_d = nc.dram_tensor("wp_l", [128, KC_D, D], f32r, kind="ExternalInput").ap()
    w1_d = nc.dram_tensor("w1_l", [128, KC_D, DH], f32r, kind="ExternalInput").ap()
    w2_d = nc.dram_tensor("w2_l", [128, KC_H, D], f32r, kind="ExternalInput").ap()
    bq_d = nc.dram_tensor("bq_l", [128, KC_D], f32, kind="ExternalInput").ap()
    bk_d = nc.dram_tensor("bk_l", [128, KC_D], f32, kind="ExternalInput").ap()
    b1_d = nc.dram_tensor("b1_l", [128, KC_H], f32, kind="ExternalInput").ap()
    mask_d = nc.dram_tensor("mask", [T, T], f32, kind="ExternalInput").ap()
    id_d = nc.dram_tensor("ident", [128, 128], f32, kind="ExternalInput").ap()
    bv_d = bp_d = b2_d = None
    if use_bv:
        bv_d = nc.dram_tensor("bv_bc", [128, D], f32, kind="ExternalInput").ap()
    if use_bp:
        bp_d = nc.dram_tensor("bp_bc", [128, D], f32, kind="ExternalInput").ap()
    if use_b2:
        b2_d = nc.dram_tensor("b2_bc", [128, D], f32, kind="ExternalInput").ap()
    out_d = nc.dram_tensor("out", [BC, T, D], f32, kind="ExternalOutput").ap()

    with tile.TileContext(nc) as tc, ExitStack() as ctx:
        wpool = ctx.enter_context(tc.tile_pool(name="wpool", bufs=1))
        px = ctx.enter_context(tc.tile_pool(name="px", bufs=8))
        pxn = ctx.enter_context(tc.tile_pool(name="pxn", bufs=2))
        pst = ctx.enter_context(tc.tile_pool(name="pst", bufs=6))
        pxt = ctx.enter_context(tc.tile_pool(name="pxt", bufs=2))
        pqk = ctx.enter_context(tc.tile_pool(name="pqk", bufs=1))
        pv = ctx.enter_context(tc.tile_pool(name="pv", bufs=1))
        patt = ctx.enter_context(tc.tile_pool(name="patt", bufs=3))
        pot = ctx.enter_context(tc.tile_pool(name="pot", bufs=1))
        px2 = ctx.enter_context(tc.tile_pool(name="px2", bufs=1))
        phid = ctx.enter_context(tc.tile_pool(name="phid", bufs=1))
        pout = ctx.enter_context(tc.tile_pool(name="pout", bufs=3))
        psq = ctx.enter_context(tc.tile_pool(name="psq", bufs=2, space="PSUM"))
        pstk = ctx.enter_context(tc.tile_pool(name="pstk", bufs=2, space="PSUM"))
        psa = ctx.enter_context(tc.tile_pool(name="psa", bufs=4, space="PSUM"))

        # resident weights / constants
        wq_sb = wpool.tile([128, KC_D, D], f32r)
        nc.sync.dma_start(out=wq_sb[:], in_=wq_d[:])
        wk_sb = wpool.tile([128, KC_D, D], f32r)
        nc.sync.dma_start(out=wk_sb[:], in_=wk_d[:])
        wv_sb = wpool.tile([128, KC_D, D], f32r)
        nc.sync.dma_start(out=wv_sb[:], in_=wv_d[:])
        wp_sb = wpool.tile([128, KC_D, D], f32r)
        nc.sync.dma_start(out=wp_sb[:], in_=wp_d[:])
        w1_sb = wpool.tile([128, KC_D, DH], f32r)
        nc.sync.dma_start(out=w1_sb[:], in_=w1_d[:])
        w2_sb = wpool.tile([128, KC_H, D], f32r)
        nc.sync.dma_start(out=w2_sb[:], in_=w2_d[:])
        bq_sb = wpool.tile([128, KC_D], f32)
        nc.sync.dma_start(out=bq_sb[:], in_=bq_d[:])
        bk_sb = wpool.tile([128, KC_D], f32)
        nc.sync.dma_start(out=bk_sb[:], in_=bk_d[:])
        b1_sb = wpool.tile([128, KC_H], f32)
        nc.sync.dma_start(out=b1_sb[:], in_=b1_d[:])
        mask_sb = wpool.tile([T, T], f32)
        nc.sync.dma_start(out=mask_sb[:], in_=mask_d[:])
        id_sb = wpool.tile([128, 128], f32)
        nc.sync.dma_start(out=id_sb[:], in_=id_d[:])
        eps_sb = wpool.tile([128, 1], f32)
        nc.vector.memset(eps_sb[:], EPS)
        bv_sb = bp_sb = b2_sb = None
        if use_bv:
            bv_sb = wpool.tile([128, D], f32)
            nc.sync.dma_start(out=bv_sb[:], in_=bv_d[:])
        if use_bp:
            bp_sb = wpool.tile([128, D], f32)
            nc.sync.dma_start(out=bp_sb[:], in_=bp_d[:])
        if use_b2:
            b2_sb = wpool.tile([128, D], f32)
            nc.sync.dma_start(out=b2_sb[:], in_=b2_d[:])

        assert D <= nc.vector.BN_STATS_FMAX

        def ln_normalize(x_sl, tp, tag):
            # x_sl: [tp, D] fp32 -> returns [tp, D] fp32r normalized (no affine)
            st = pst.tile([128, nc.vector.BN_STATS_DIM], f32, tag="bnst")
            nc.vector.bn_stats(out=st[:tp], in_=x_sl)
            mv = pst.tile([128, nc.vector.BN_AGGR_DIM], f32, tag="bnmv")
            nc.vector.bn_aggr(out=mv[:tp], in_=st[:tp])
            rs = pst.tile([128, 1], f32, tag="rs")
            nc.scalar.activation(out=rs[:tp], in_=mv[:tp, 1:2], func=AF.Sqrt,
                                 bias=eps_sb[:tp], scale=1.0)
            nc.vector.reciprocal(out=rs[:tp], in_=rs[:tp])
            nmu = pst.tile([128, 1], f32, tag="nmu")
            nc.vector.tensor_mul(nmu[:tp], mv[:tp, 0:1], rs[:tp])
            nc.scalar.mul(out=nmu[:tp], in_=nmu[:tp], mul=-1.0)
            xn = pxn.tile([128, D], f32, tag=tag)
            nc.scalar.activation(out=xn[:tp], in_=x_sl, func=AF.Identity,
                                 bias=nmu[:tp], scale=rs[:tp])
            return xn

        xf = x_d.flatten_outer_dims()
        of = out_d.flatten_outer_dims()

        groups = []
        tok = 0
        nbat = BC
        while nbat > 0:
            nb = min(NB, nbat)
            groups.append((tok, nb))
            tok += nb * T
            nbat -= nb

        for (tok0, nb) in groups:
            ntok = nb * T
            tp = ntok // 4

            # --- A: load x, LN1, transpose to feature-major XnT ---
            XnT = pxt.tile([128, KC_D, NTOK_MAX], f32r, tag="xnt")
            xres = []
            for i in range(4):
                xt_ = px.tile([128, D], f32, tag="xres")
                nc.sync.dma_start(out=xt_[:tp], in_=xf[tok0 + i * tp: tok0 + (i + 1) * tp])
                xres.append(xt_)
                xn = ln_normalize(xt_[:tp], tp, "xn1")
                for c in range(KC_D):
                    pstr = psa.tile([128, 128], f32, tag="ps_small")
                    nc.tensor.transpose(pstr[:128, :tp], xn[:tp, ts(c, 128)], id_sb[:tp, :tp])
                    nc.vector.tensor_copy(XnT[:, c, i * tp:(i + 1) * tp], pstr[:128, :tp])

            # --- B: Q^T, K^T (feature-major) ---
            QT = pqk.tile([128, KC_D, NTOK_MAX], f32r, tag="qt")
            KT = pqk.tile([128, KC_D, NTOK_MAX], f32r, tag="kt")
            for dst, w_sb, b_sb in ((QT, wq_sb, bq_sb), (KT, wk_sb, bk_sb)):
                for m in range(KC_D):
                    ps = psq.tile([128, NTOK_MAX], f32, tag="ps_big")
                    for kc in range(KC_D):
                        nc.tensor.matmul(ps[:, :ntok], w_sb[:, kc, ts(m, 128)],
                                         XnT[:, kc, :ntok],
                                         start=(kc == 0), stop=(kc == KC_D - 1))
                    nc.scalar.activation(out=dst[:, m, :ntok], in_=ps[:, :ntok],
                                         func=AF.Identity, bias=b_sb[:, m:m + 1], scale=1.0)

            # --- C: V (token-major, per batch) ---
            V = pv.tile([128, NB, D], f32, tag="v")
            for b in range(nb):
                ps = pstk.tile([128, D], f32, tag="ps_tok")
                for kc in range(KC_D):
                    nc.tensor.matmul(ps[:T, :], XnT[:, kc, b * T:(b + 1) * T],
                                     wv_sb[:, kc, :],
                                     start=(kc == 0), stop=(kc == KC_D - 1))
                if use_bv:
                    nc.vector.tensor_add(V[:T, b, :], ps[:T, :], bv_sb[:T, :])
                else:
                    nc.vector.tensor_copy(V[:T, b, :], ps[:T, :])

            # --- D: attention per (batch, head) -> feature-major OT ---
            OT = pot.tile([128, KC_D, NTOK_MAX], f32r, tag="ot")
            for b in range(nb):
                for h in range(H):
                    mb, mo = h // 2, (h % 2) * 64
                    bs = slice(b * T, (b + 1) * T)
                    ps_s = psa.tile([128, 128], f32, tag="ps_small")
                    nc.tensor.matmul(ps_s[:T, :T], QT[mo:mo + 64, mb, bs],
                                     KT[mo:mo + 64, mb, bs], start=True, stop=True)
                    sm = patt.tile([128, 128], f32, tag="sm")
                    nc.vector.tensor_add(sm[:T, :T], ps_s[:T, :T], mask_sb[:T, :T])
                    wei = patt.tile([128, 128], f32, tag="wei")
                    den = pst.tile([128, 1], f32, tag="den")
                    nc.scalar.activation(out=wei[:T, :T], in_=sm[:T, :T], func=AF.Exp,
                                         accum_out=den[:T])
                    rden = pst.tile([128, 1], f32, tag="rden")
                    nc.vector.reciprocal(out=rden[:T], in_=den[:T])
                    weis = patt.tile([128, 128], f32, tag="weis")
                    nc.scalar.activation(out=weis[:T, :T], in_=wei[:T, :T],
                                         func=AF.Identity, scale=rden[:T])
                    ps_t = psa.tile([128, 128], f32, tag="ps_small")
                    nc.tensor.transpose(ps_t[:T, :T], weis[:T, :T], id_sb[:T, :T])
                    weiT = patt.tile([128, 128], f32, tag="weiT")
                    nc.vector.tensor_copy(weiT[:T, :T], ps_t[:T, :T])
                    ps_o = psa.tile([128, 128], f32, tag="ps_small")
                    nc.tensor.matmul(ps_o[:64, :T], V[:T, b, ts(h, 64)], weiT[:T, :T],
                                     start=True, stop=True)
                    nc.scalar.copy(out=OT[mo:mo + 64, mb, bs], in_=ps_o[:64, :T])

            # --- E: attention out projection + residual -> X2 (token-major) ---
            X2 = px2.tile([128, 4, D], f32, tag="x2")
            for i in range(4):
                ps = pstk.tile([128, D], f32, tag="ps_tok")
                for kc in range(KC_D):
                    nc.tensor.matmul(ps[:tp, :], OT[:, kc, i * tp:(i + 1) * tp],
                                     wp_sb[:, kc, :],
                                     start=(kc == 0), stop=(kc == KC_D - 1))
                if use_bp:
                    nc.vector.tensor_add(ps[:tp, :], ps[:tp, :], bp_sb[:tp, :])
                nc.vector.tensor_add(X2[:tp, i, :], ps[:tp, :], xres[i][:tp, :])

            # --- F: LN2 + transpose ---
            Xn2T = pxt.tile([128, KC_D, NTOK_MAX], f32r, tag="xn2t")
            for i in range(4):
                xn2 = ln_normalize(X2[:tp, i, :], tp, "xn2")
                for c in range(KC_D):
                    pstr = psa.tile([128, 128], f32, tag="ps_small")
                    nc.tensor.transpose(pstr[:128, :tp], xn2[:tp, ts(c, 128)], id_sb[:tp, :tp])
                    nc.vector.tensor_copy(Xn2T[:, c, i * tp:(i + 1) * tp], pstr[:128, :tp])

            # --- G: FFN1 -> relu -> feature-major HT ---
            HT = phid.tile([128, KC_H, NTOK_MAX], f32r, tag="hid")
            for m in range(KC_H):
                ps = psq.tile([128, NTOK_MAX], f32, tag="ps_big")
                for kc in range(KC_D):
                    nc.tensor.matmul(ps[:, :ntok], w1_sb[:, kc, ts(m, 128)],
                                     Xn2T[:, kc, :ntok],
                                     start=(kc == 0), stop=(kc == KC_D - 1))
                nc.scalar.activation(out=HT[:, m, :ntok], in_=ps[:, :ntok], func=AF.Relu,
                                     bias=b1_sb[:, m:m + 1], scale=1.0)

            # --- H: FFN2 + residual -> out ---
            for i in range(4):
                ps = pstk.tile([128, D], f32, tag="ps_tok")
                for kc in range(KC_H):
                    nc.tensor.matmul(ps[:tp, :], HT[:, kc, i * tp:(i + 1) * tp],
                                     w2_sb[:, kc, :],
                                     start=(kc == 0), stop=(kc == KC_H - 1))
                if use_b2:
                    nc.vector.tensor_add(ps[:tp, :], ps[:tp, :], b2_sb[:tp, :])
                ot_ = pout.tile([128, D], f32, tag="outt")
                nc.vector.tensor_add(ot_[:tp, :], ps[:tp, :], X2[:tp, i, :])
                nc.sync.dma_start(out=of[tok0 + i * tp: tok0 + (i + 1) * tp], in_=ot_[:tp, :])

    nc.compile()
    return nc


def _get_nc(use_bv, use_bp, use_b2):
    key = (use_bv, use_bp, use_b2)
    if key not in _NC_CACHE:
        _NC_CACHE[key] = _build_nc(*key)
    return _NC_CACHE[key]


def _prep_inputs(x, wq, wk, wv, wproj, bproj, w1, b1, w2, b2, g1, beta1, g2, beta2):
    f = np.float32
    # stack per-head projections into [D, D] with head h at columns h*HS:(h+1)*HS
    wq_f = np.ascontiguousarray(wq.transpose(1, 0, 2).reshape(D, D), dtype=f)
    wk_f = np.ascontiguousarray(wk.transpose(1, 0, 2).reshape(D, D), dtype=f)
    wv_f = np.ascontiguousarray(wv.transpose(1, 0, 2).reshape(D, D), dtype=f)
    scale = np.float32(HS ** 0.5)
    # fold LN1 affine into qkv weights, LN2 affine into w1
    wq_p = (g1[:, None] * wq_f) * scale
    wk_p = g1[:, None] * wk_f
    wv_p = g1[:, None] * wv_f
    w1_p = g2[:, None] * w1
    bq = (beta1 @ wq_f) * scale
    bk = beta1 @ wk_f
    bv = beta1 @ wv_f
    b1_p = b1 + beta2 @ w1
    bp = bproj
    b2_p = b2

    def lay(w, kc):
        # [K, M] -> [128, kc, M] with K split into kc chunks of 128
        return np.ascontiguousarray(
            w.reshape(kc, 128, w.shape[1]).transpose(1, 0, 2), dtype=f)

    def layb(bias, kc):
        return np.ascontiguousarray(bias.reshape(kc, 128).T, dtype=f)

    mask = np.zeros((T, T), dtype=f)
    mask[np.triu_indices(T, 1)] = MASK_VAL

    shared = {
        "wq_l": lay(wq_p, KC_D), "wk_l": lay(wk_p, KC_D), "wv_l": lay(wv_p, KC_D),
        "wp_l": lay(np.asarray(wproj, dtype=f), KC_D), "w1_l": lay(w1_p, KC_D),
        "w2_l": lay(np.asarray(w2, dtype=f), KC_H),
        "bq_l": layb(bq, KC_D), "bk_l": layb(bk, KC_D), "b1_l": layb(b1_p, KC_H),
        "mask": mask, "ident": np.eye(128, dtype=f),
    }
    use_bv = bool(np.any(bv))
    use_bp = bool(np.any(bp))
    use_b2 = bool(np.any(b2_p))
    if use_bv:
        shared["bv_bc"] = np.ascontiguousarray(np.tile(bv.astype(f), (128, 1)))
    if use_bp:
        shared["bp_bc"] = np.ascontiguousarray(np.tile(np.asarray(bp, f), (128, 1)))
    if use_b2:
        shared["b2_bc"] = np.ascontiguousarray(np.tile(np.asarray(b2_p, f), (128, 1)))
    return shared, (use_bv, use_bp, use_b2)


def kernel(**inputs):
    from concourse.bass_utils import run_bass_kernel_spmd

    x = np.asarray(inputs["x"], dtype=np.float32)
    shared, flags = _prep_inputs(
        x, *[np.asarray(inputs[k], dtype=np.float32) for k in
             ("wq", "wk", "wv", "wproj", "bproj", "w1", "b1", "w2", "b2",
              "g1", "beta1", "g2", "beta2")])
    nc = _get_nc(*flags)
    in_maps = []
    for c in range(N_CORES):
        m = dict(shared)
        m["x"] = np.ascontiguousarray(x[c * BC:(c + 1) * BC])
        in_maps.append(m)
    res = run_bass_kernel_spmd(nc, in_maps, core_ids=list(range(N_CORES)))
    return np.concatenate([res.results[i]["out"] for i in range(N_CORES)], axis=0)


# revision 14
# speedup vs baseline: 1.9227x; 1.9227x over previous
"""Trainium2 Bass kernel for a dense pre-norm transformer block.

Reference semantics (per batch b, seq len T=100, d_model D=384, heads H=6):
  h   = LN(x) * g1 + beta1
  q,k,v = per-head projections of h;  wei = softmax(mask(q k^T * sqrt(64)))
  x2  = x + (wei v) Wp + bp
  out = x2 + relu(LN(x2)*g2+beta2 @ W1 + b1) @ W2 + b2

Distribution: data-parallel over the batch dim across 8 NeuronCores
(64 batches each); weights are replicated. No collectives.

Host-side folds: LN affines are folded into the adjacent matmul
weights/biases; the sqrt(head_size) score scale is folded into Wq.
Big matmuls run as float32r (reduced-precision fp32 at full PE rate).

Attention avoids all per-head transposes: scores are computed
transposed ([u,t] = K^T-slice stationary x Q^T-slice moving), the
softmax denominator comes from ones-columns appended to V inside the
same AV matmul, and the 1/den scaling happens in token-major layout
where it is a per-partition scalar.
"""

import numpy as np
from contextlib import ExitStack

B, T, D = 512, 100, 384
H, HS = 6, 64
DH = 4 * D                      # FFN hidden 1536
N_CORES = 8
BC = B // N_CORES               # batches per core
EPS = 1e-5
MASK_VAL = -1e9
NB = 5                          # batches per inner group
NTOK_MAX = NB * T               # 500
KC_D = D // 128                 # 3 contraction chunks over D
KC_H = DH // 128                # 12 contraction chunks over DH
VW = 66                         # V columns per head: 64 + 2 ones (denominator)

_NC_CACHE = {}
SPLIT_S = False
SPLIT_A = False
SPLIT_T = False


def _build_nc(use_bv, use_bp, use_b2):
    import concourse.bass as bass
    import concourse.tile as tile
    from concourse import bacc, mybir

    f32 = mybir.dt.float32
    f32r = mybir.dt.float32r
    AF = mybir.ActivationFunctionType
    OP = mybir.AluOpType
    ts = bass.ts

    nc = bacc.Bacc("TRN2", target_bir_lowering=False, debug=False,
                   enable_asserts=True, num_devices=N_CORES)

    x_d = nc.dram_tensor("x", [BC, T, D], f32, kind="ExternalInput").ap()
    wq_d = nc.dram_tensor("wq_l", [128, KC_D, D], f32r, kind="ExternalInput").ap()
    wk_d = nc.dram_tensor("wk_l", [128, KC_D, D], f32r, kind="ExternalInput").ap()
    wv_d = nc.dram_tensor("wv_l", [128, KC_D, D], f32r, kind="ExternalInput").ap()
    wp_d = nc.dram_tensor("wp_l", [128, KC_D, D], f32r, kind="ExternalInput").ap()
    w1_d = nc.dram_tensor("w1_l", [128, KC_D, DH], f32r, kind="ExternalInput").ap()
    w2_d = nc.dram_tensor("w2_l", [128, KC_H, D], f32r, kind="ExternalInput").ap()
    bq_d = nc.dram_tensor("bq_l", [128, KC_D], f32, kind="ExternalInput").ap()
    bk_d = nc.dram_tensor("bk_l", [128, KC_D], f32, kind="ExternalInput").ap()
    b1_d = nc.dram_tensor("b1_l", [128, KC_H], f32, kind="ExternalInput").ap()
    mask_d = nc.dram_tensor("mask3", [T, 3 * T], f32, kind="ExternalInput").ap()
    id_d = nc.dram_tensor("ident", [128, 128], f32, kind="ExternalInput").ap()
    bv_d = bp_d = b2_d = None
    if use_bv:
        bv_d = nc.dram_tensor("bv_bc", [128, D], f32, kind="ExternalInput").ap()
    if use_bp:
        bp_d = nc.dram_tensor("bp_bc", [128, D], f32, kind="ExternalInput").ap()
    if use_b2:
        b2_d = nc.dram_tensor("b2_bc", [128, D], f32, kind="ExternalInput").ap()
    out_d = nc.dram_tensor("out", [BC, T, D], f32, kind="ExternalOutput").ap()

    with tile.TileContext(nc) as tc, ExitStack() as ctx:
        wpool = ctx.enter_context(tc.tile_pool(name="wpool", bufs=1))
        px = ctx.enter_context(tc.tile_pool(name="px", bufs=8))
        pxn = ctx.enter_context(tc.tile_pool(name="pxn", bufs=2))
        pst = ctx.enter_context(tc.tile_pool(name="pst", bufs=6))
        pxt = ctx.enter_context(tc.tile_pool(name="pxt", bufs=2))
        pqk = ctx.enter_context(tc.tile_pool(name="pqk", bufs=1))
        pv = ctx.enter_context(tc.tile_pool(name="pv", bufs=1))
        patt = ctx.enter_context(tc.tile_pool(name="patt", bufs=3))
        po = ctx.enter_context(tc.tile_pool(name="po", bufs=2))
        pot = ctx.enter_context(tc.tile_pool(name="pot", bufs=1))
        px2 = ctx.enter_context(tc.tile_pool(name="px2", bufs=1))
        phid = ctx.enter_context(tc.tile_pool(name="phid", bufs=1))
        pout = ctx.enter_context(tc.tile_pool(name="pout", bufs=3))
        # PSUM: 8 banks total -> (pool,tag) bufs must sum to <= 8
        psb = 1 if (SPLIT_S or SPLIT_A or SPLIT_T) else 2
        psq = ctx.enter_context(tc.tile_pool(name="psq", bufs=psb, space="PSUM"))
        pstk = ctx.enter_context(tc.tile_pool(name="pstk", bufs=psb, space="PSUM"))
        psa = ctx.enter_context(tc.tile_pool(name="psa", bufs=psb, space="PSUM"))
        psv = ctx.enter_context(tc.tile_pool(name="psv", bufs=psb, space="PSUM"))

        # resident weights / constants
        wq_sb = wpool.tile([128, KC_D, D], f32r)
        nc.sync.dma_start(out=wq_sb[:], in_=wq_d[:])
        wk_sb = wpool.tile([128, KC_D, D], f32r)
        nc.sync.dma_start(out=wk_sb[:], in_=wk_d[:])
        wv_sb = wpool.tile([128, KC_D, D], f32r)
        nc.sync.dma_start(out=wv_sb[:], in_=wv_d[:])
        wp_sb = wpool.tile([128, KC_D, D], f32r)
        nc.sync.dma_start(out=wp_sb[:], in_=wp_d[:])
        w1_sb = wpool.tile([128, KC_D, DH], f32r)
        nc.sync.dma_start(out=w1_sb[:], in_=w1_d[:])
        w2_sb = wpool.tile([128, KC_H, D], f32r)
        nc.sync.dma_start(out=w2_sb[:], in_=w2_d[:])
        bq_sb = wpool.tile([128, KC_D], f32)
        nc.sync.dma_start(out=bq_sb[:], in_=bq_d[:])
        bk_sb = wpool.tile([128, KC_D], f32)
        nc.sync.dma_start(out=bk_sb[:], in_=bk_d[:])
        b1_sb = wpool.tile([128, KC_H], f32)
        nc.sync.dma_start(out=b1_sb[:], in_=b1_d[:])
        mask_sb = wpool.tile([T, 3 * T], f32)
        nc.sync.dma_start(out=mask_sb[:], in_=mask_d[:])
        id_sb = wpool.tile([128, 128], f32)
        nc.sync.dma_start(out=id_sb[:], in_=id_d[:])
        eps_sb = wpool.tile([128, 1], f32)
        nc.vector.memset(eps_sb[:], EPS)
        bv_sb = bp_sb = b2_sb = None
        if use_bv:
            bv_sb = wpool.tile([128, D], f32)
            nc.sync.dma_start(out=bv_sb[:], in_=bv_d[:])
        if use_bp:
            bp_sb = wpool.tile([128, D], f32)
            nc.sync.dma_start(out=bp_sb[:], in_=bp_d[:])
        if use_b2:
            b2_sb = wpool.tile([128, D], f32)
            nc.sync.dma_start(out=b2_sb[:], in_=b2_d[:])

        assert D <= nc.vector.BN_STATS_FMAX

        def ln_normalize(x_sl, tp, tag):
            # x_sl: [tp, D] fp32 -> [tp, D] fp32 normalized (no affine)
            st = pst.tile([128, nc.vector.BN_STATS_DIM], f32, tag="bnst")
            nc.vector.bn_stats(out=st[:tp], in_=x_sl)
            mv = pst.tile([128, nc.vector.BN_AGGR_DIM], f32, tag="bnmv")
            nc.vector.bn_aggr(out=mv[:tp], in_=st[:tp])
            rs = pst.tile([128, 1], f32, tag="rs")
            nc.scalar.activation(out=rs[:tp], in_=mv[:tp, 1:2], func=AF.Sqrt,
                                 bias=eps_sb[:tp], scale=1.0)
            nc.vector.reciprocal(out=rs[:tp], in_=rs[:tp])
            nmu = pst.tile([128, 1], f32, tag="nmu")
            nc.vector.tensor_scalar(out=nmu[:tp], in0=mv[:tp, 0:1], scalar1=rs[:tp],
                                    scalar2=-1.0, op0=OP.mult, op1=OP.mult)
            xn = pxn.tile([128, D], f32, tag=tag)
            nc.scalar.activation(out=xn[:tp], in_=x_sl, func=AF.Identity,
                                 bias=nmu[:tp], scale=rs[:tp])
            return xn

        def ln_transpose(xn, dstT, tp, ioff):
            # xn [tp, D] -> dstT[:, c, ioff:ioff+tp] (feature-major, f32r)
            if SPLIT_T:
                for c in range(KC_D):
                    psc = psq.tile([128, 128], f32, tag="ps_big")
                    nc.tensor.transpose(psc[:128, :tp],
                                        xn[:tp, ts(c, 128)], id_sb[:tp, :tp])
                    nc.vector.tensor_copy(dstT[:, c, ioff:ioff + tp], psc[:, :tp])
            else:
                ps = psq.tile([128, KC_D, 128], f32, tag="ps_big")
                for c in range(KC_D):
                    nc.tensor.transpose(ps[:128, c, :tp],
                                        xn[:tp, ts(c, 128)], id_sb[:tp, :tp])
                nc.vector.tensor_copy(dstT[:, :, ioff:ioff + tp], ps[:, :, :tp])

        xf = x_d.flatten_outer_dims()
        of = out_d.flatten_outer_dims()

        groups = []
        tok = 0
        nbat = BC
        while nbat > 0:
            nb = min(NB, nbat)
            groups.append((tok, nb))
            tok += nb * T
            nbat -= nb

        for (tok0, nb) in groups:
            ntok = nb * T
            tp = ntok // 4

            # --- A: load x, LN1, transpose to feature-major XnT ---
            XnT = pxt.tile([128, KC_D, NTOK_MAX], f32r, tag="xnt")
            xres = []
            for i in range(4):
                xt_ = px.tile([128, D], f32, tag="xres")
                nc.sync.dma_start(out=xt_[:tp], in_=xf[tok0 + i * tp: tok0 + (i + 1) * tp])
                xres.append(xt_)
                xn = ln_normalize(xt_[:tp], tp, "xn1")
                ln_transpose(xn, XnT, tp, i * tp)

            # --- B: Q^T, K^T (feature-major) ---
            QT = pqk.tile([128, KC_D, NTOK_MAX], f32, tag="qt")
            KT = pqk.tile([128, KC_D, NTOK_MAX], f32, tag="kt")
            for dst, w_sb, b_sb in ((QT, wq_sb, bq_sb), (KT, wk_sb, bk_sb)):
                for m in range(KC_D):
                    ps = psq.tile([128, NTOK_MAX], f32, tag="ps_big")
                    for kc in range(KC_D):
                        nc.tensor.matmul(ps[:, :ntok], w_sb[:, kc, ts(m, 128)],
                                         XnT[:, kc, :ntok],
                                         start=(kc == 0), stop=(kc == KC_D - 1))
                    nc.vector.tensor_scalar_add(out=dst[:, m, :ntok], in0=ps[:, :ntok],
                                                scalar1=b_sb[:, m:m + 1])

            # --- C: V (token-major, per batch, 66-wide per head w/ ones cols) ---
            V = pv.tile([128, NB, H, VW], f32, tag="v")
            nc.vector.memset(V[:T, :nb, :, 64:VW], 1.0)
            for b in range(nb):
                ps = pstk.tile([128, D], f32, tag="ps_tok")
                for kc in range(KC_D):
                    nc.tensor.matmul(ps[:T, :], XnT[:, kc, b * T:(b + 1) * T],
                                     wv_sb[:, kc, :],
                                     start=(kc == 0), stop=(kc == KC_D - 1))
                psh = ps[:T].rearrange("p (h s) -> p h s", h=H)
                if use_bv:
                    bvh = bv_sb[:T].rearrange("p (h s) -> p h s", h=H)
                    nc.vector.tensor_add(V[:T, b, :, 0:64], psh, bvh)
                else:
                    nc.vector.tensor_copy(V[:T, b, :, 0:64], psh)

            # --- D: attention -> token-major O -> feature-major OT ---
            OT = pot.tile([128, KC_D, NTOK_MAX], f32r, tag="ot")
            for b in range(nb):
                bs = slice(b * T, (b + 1) * T)
                O_sb = po.tile([128, D], f32, tag="o")
                rden = pst.tile([128, H], f32, tag="rden")
                for g3 in range(2):          # two triples of heads
                    hs3 = [g3, g3 + 2, g3 + 4]   # same (h%2): one PE row-group per triple
                    if SPLIT_S:
                        ps_ss = [psa.tile([128, T], f32, tag=f"ps_att{j}", name=f"ps_s{j}") for j in range(3)]
                    else:
                        ps_s = psa.tile([128, 3, T], f32, tag="ps_att")
                    for j, h in enumerate(hs3):
                        mb, mo = h // 2, (h % 2) * 64
                        # scores^T[u,t] for head h
                        nc.tensor.matmul(ps_ss[j][:T, :] if SPLIT_S else ps_s[:T, j, :],
                                         KT[mo:mo + 64, mb, bs],
                                         QT[mo:mo + 64, mb, bs],
                                         start=True, stop=True)
                    sm = patt.tile([128, 3 * T], f32, tag="sm")
                    if SPLIT_S:
                        for j in range(3):
                            nc.vector.tensor_add(sm[:T, j * T:(j + 1) * T],
                                                 ps_ss[j][:T, :], mask_sb[:T, j * T:(j + 1) * T])
                    else:
                        nc.vector.tensor_add(sm[:T], ps_s[:T, :, :], mask_sb[:T])
                    ex = patt.tile([128, 3 * T], f32, tag="ex")
                    nc.scalar.activation(out=ex[:T], in_=sm[:T], func=AF.Exp)
                    if SPLIT_A:
                        ps_os = [psv.tile([128, VW], f32, tag=f"ps_av{j}", name=f"ps_o{j}") for j in range(3)]
                        for j, h in enumerate(hs3):
                            nc.tensor.matmul(ps_os[j][:T, :],
                                             ex[:T, ts(j, T)],
                                             V[:T, b, h, :],
                                             start=True, stop=True)
                        for j, h in enumerate(hs3):
                            nc.vector.reciprocal(out=rden[:T, h:h + 1],
                                                 in_=ps_os[j][:T, 64:65])
                            nc.vector.tensor_scalar_mul(out=O_sb[:T, ts(h, 64)],
                                                        in0=ps_os[j][:T, 0:64],
                                                        scalar1=rden[:T, h:h + 1])
                    else:
                        ps_o = psv.tile([128, 3, VW], f32, tag="ps_av")
                        for j, h in enumerate(hs3):
                            nc.tensor.matmul(ps_o[:T, j, :],
                                             ex[:T, ts(j, T)],
                                             V[:T, b, h, :],
                                             start=True, stop=True)
                        nc.vector.reciprocal(out=rden[:T, g3 * 3:(g3 + 1) * 3],
                                             in_=ps_o[:T, :, 64:65])
                        for j, h in enumerate(hs3):
                            nc.vector.tensor_scalar_mul(out=O_sb[:T, ts(h, 64)],
                                                        in0=ps_o[:T, j, 0:64],
                                                        scalar1=rden[:T, g3 * 3 + j:g3 * 3 + j + 1])
                # transpose O into feature-major OT
                if SPLIT_T:
                    for c in range(KC_D):
                        ps_tc = psa.tile([128, T], f32, tag=f"ps_att{c}")
                        nc.tensor.transpose(ps_tc[:128, :],
                                            O_sb[:T, ts(c, 128)], id_sb[:T, :T])
                        nc.vector.tensor_copy(OT[:, c, bs], ps_tc[:, :])
                else:
                    ps_t = psa.tile([128, KC_D, T], f32, tag="ps_att")
                    for c in range(KC_D):
                        nc.tensor.transpose(ps_t[:128, c, :],
                                            O_sb[:T, ts(c, 128)], id_sb[:T, :T])
                    nc.vector.tensor_copy(OT[:, :, bs], ps_t[:, :, :])

            # --- E: attention out projection + residual -> X2 (token-major) ---
            X2 = px2.tile([128, 4, D], f32, tag="x2")
            for i in range(4):
                ps = pstk.tile([128, D], f32, tag="ps_tok")
                for kc in range(KC_D):
                    nc.tensor.matmul(ps[:tp, :], OT[:, kc, i * tp:(i + 1) * tp],
                                     wp_sb[:, kc, :],
                                     start=(kc == 0), stop=(kc == KC_D - 1))
                if use_bp:
                    nc.vector.tensor_add(ps[:tp, :], ps[:tp, :], bp_sb[:tp, :])
                nc.vector.tensor_add(X2[:tp, i, :], ps[:tp, :], xres[i][:tp, :])

            # --- F: LN2 + transpose ---
            Xn2T = pxt.tile([128, KC_D, NTOK_MAX], f32r, tag="xn2t")
            for i in range(4):
                xn2 = ln_normalize(X2[:tp, i, :], tp, "xn2")
                ln_transpose(xn2, Xn2T, tp, i * tp)

            # --- G: FFN1 -> relu(+bias) -> feature-major HT ---
            HT = phid.tile([128, KC_H, NTOK_MAX], f32r, tag="hid")
            for m in range(KC_H):
                ps = psq.tile([128, NTOK_MAX], f32, tag="ps_big")
                for kc in range(KC_D):
                    nc.tensor.matmul(ps[:, :ntok], w1_sb[:, kc, ts(m, 128)],
                                     Xn2T[:, kc, :ntok],
                                     start=(kc == 0), stop=(kc == KC_D - 1))
                nc.vector.tensor_scalar(out=HT[:, m, :ntok], in0=ps[:, :ntok],
                                        scalar1=b1_sb[:, m:m + 1], scalar2=0.0,
                                        op0=OP.add, op1=OP.max)

            # --- H: FFN2 + residual -> out ---
            for i in range(4):
                ps = pstk.tile([128, D], f32, tag="ps_tok")
                for kc in range(KC_H):
                    nc.tensor.matmul(ps[:tp, :], HT[:, kc, i * tp:(i + 1) * tp],
                                     w2_sb[:, kc, :],
                                     start=(kc == 0), stop=(kc == KC_H - 1))
                if use_b2:
                    nc.vector.tensor_add(ps[:tp, :], ps[:tp, :], b2_sb[:tp, :])
                ot_ = pout.tile([128, D], f32, tag="outt")
                nc.vector.tensor_add(ot_[:tp, :], ps[:tp, :], X2[:tp, i, :])
                nc.sync.dma_start(out=of[tok0 + i * tp: tok0 + (i + 1) * tp], in_=ot_[:tp, :])

    nc.compile()
    return nc


def _get_nc(use_bv, use_bp, use_b2):
    key = (use_bv, use_bp, use_b2)
    if key not in _NC_CACHE:
        _NC_CACHE[key] = _build_nc(*key)
    return _NC_CACHE[key]


def _prep_inputs(x, wq, wk, wv, wproj, bproj, w1, b1, w2, b2, g1, beta1, g2, beta2):
    f = np.float32
    # stack per-head projections into [D, D] with head h at columns h*HS:(h+1)*HS
    wq_f = np.ascontiguousarray(wq.transpose(1, 0, 2).reshape(D, D), dtype=f)
    wk_f = np.ascontiguousarray(wk.transpose(1, 0, 2).reshape(D, D), dtype=f)
    wv_f = np.ascontiguousarray(wv.transpose(1, 0, 2).reshape(D, D), dtype=f)
    scale = np.float32(HS ** 0.5)
    # fold LN1 affine into qkv weights, LN2 affine into w1
    wq_p = (g1[:, None] * wq_f) * scale
    wk_p = g1[:, None] * wk_f
    wv_p = g1[:, None] * wv_f
    w1_p = g2[:, None] * w1
    bq = (beta1 @ wq_f) * scale
    bk = beta1 @ wk_f
    bv = beta1 @ wv_f
    b1_p = b1 + beta2 @ w1
    bp = bproj
    b2_p = b2

    def lay(w, kc):
        # [K, M] -> [128, kc, M] with K split into kc chunks of 128
        return np.ascontiguousarray(
            w.reshape(kc, 128, w.shape[1]).transpose(1, 0, 2), dtype=f)

    def layb(bias, kc):
        return np.ascontiguousarray(bias.reshape(kc, 128).T, dtype=f)

    # transposed causal mask, tiled for 3 heads: keep (t >= u)
    maskT = np.full((T, T), MASK_VAL, dtype=f)
    maskT[np.triu_indices(T)] = 0.0
    mask3 = np.ascontiguousarray(np.tile(maskT, (1, 3)))

    shared = {
        "wq_l": lay(wq_p, KC_D), "wk_l": lay(wk_p, KC_D), "wv_l": lay(wv_p, KC_D),
        "wp_l": lay(np.asarray(wproj, dtype=f), KC_D), "w1_l": lay(w1_p, KC_D),
        "w2_l": lay(np.asarray(w2, dtype=f), KC_H),
        "bq_l": layb(bq, KC_D), "bk_l": layb(bk, KC_D), "b1_l": layb(b1_p, KC_H),
        "mask3": mask3, "ident": np.eye(128, dtype=f),
    }
    use_bv = bool(np.any(bv))
    use_bp = bool(np.any(bp))
    use_b2 = bool(np.any(b2_p))
    if use_bv:
        shared["bv_bc"] = np.ascontiguousarray(np.tile(bv.astype(f), (128, 1)))
    if use_bp:
        shared["bp_bc"] = np.ascontiguousarray(np.tile(np.asarray(bp, f), (128, 1)))
    if use_b2:
        shared["b2_bc"] = np.ascontiguousarray(np.tile(np.asarray(b2_p, f), (128, 1)))
    return shared, (use_bv, use_bp, use_b2)


def kernel(**inputs):
    from concourse.bass_utils import run_bass_kernel_spmd

    x = np.asarray(inputs["x"], dtype=np.float32)
    shared, flags = _prep_inputs(
        x, *[np.asarray(inputs[k], dtype=np.float32) for k in
             ("wq", "wk", "wv", "wproj", "bproj", "w1", "b1", "w2", "b2",
              "g1", "beta1", "g2", "beta2")])
    nc = _get_nc(*flags)
    in_maps = []
    for c in range(N_CORES):
        m = dict(shared)
        m["x"] = np.ascontiguousarray(x[c * BC:(c + 1) * BC])
        in_maps.append(m)
    res = run_bass_kernel_spmd(nc, in_maps, core_ids=list(range(N_CORES)))
    return np.concatenate([res.results[i]["out"] for i in range(N_CORES)], axis=0)


# revision 16
# speedup vs baseline: 2.5892x; 1.3466x over previous
"""Trainium2 Bass kernel for a dense pre-norm transformer block.

Reference semantics (per batch b, seq len T=100, d_model D=384, heads H=6):
  h   = LN(x) * g1 + beta1
  q,k,v = per-head projections of h;  wei = softmax(mask(q k^T * sqrt(64)))
  x2  = x + (wei v) Wp + bp
  out = x2 + relu(LN(x2)*g2+beta2 @ W1 + b1) @ W2 + b2

Distribution: data-parallel over the batch dim across 8 NeuronCores
(64 batches each); weights are replicated. No collectives.

Host-side folds: LN affines are folded into the adjacent matmul
weights/biases; the sqrt(head_size) score scale is folded into Wq.
Big matmuls run as float32r (reduced-precision fp32 at full PE rate).

Attention avoids all per-head transposes: scores are computed
transposed ([u,t] = K^T-slice stationary x Q^T-slice moving), the
softmax denominator comes from ones-columns appended to V inside the
same AV matmul, and the 1/den scaling happens in token-major layout
where it is a per-partition scalar.
"""

import numpy as np
from contextlib import ExitStack

B, T, D = 512, 100, 384
H, HS = 6, 64
DH = 4 * D                      # FFN hidden 1536
N_CORES = 8
BC = B // N_CORES               # batches per core
EPS = 1e-5
MASK_VAL = -1e9
NB = 5                          # batches per inner group
NTOK_MAX = NB * T               # 500
KC_D = D // 128                 # 3 contraction chunks over D
KC_H = DH // 128                # 12 contraction chunks over DH
VW = 66                         # V columns per head: 64 + 2 ones (denominator)

_NC_CACHE = {}


def _build_nc(use_bv, use_bp, use_b2):
    import concourse.bass as bass
    import concourse.tile as tile
    from concourse import bacc, mybir

    f32 = mybir.dt.float32
    f32r = mybir.dt.float32r
    bf16 = mybir.dt.bfloat16
    AF = mybir.ActivationFunctionType
    OP = mybir.AluOpType
    ts = bass.ts

    nc = bacc.Bacc("TRN2", target_bir_lowering=False, debug=False,
                   enable_asserts=True, num_devices=N_CORES)

    x_d = nc.dram_tensor("x", [BC, T, D], f32, kind="ExternalInput").ap()
    wq_d = nc.dram_tensor("wq_l", [128, KC_D, D], f32r, kind="ExternalInput").ap()
    wk_d = nc.dram_tensor("wk_l", [128, KC_D, D], f32r, kind="ExternalInput").ap()
    wv_d = nc.dram_tensor("wv_l", [128, KC_D, D], f32r, kind="ExternalInput").ap()
    wp_d = nc.dram_tensor("wp_l", [128, KC_D, D], f32r, kind="ExternalInput").ap()
    w1_d = nc.dram_tensor("w1_l", [128, KC_D, DH], f32r, kind="ExternalInput").ap()
    w2_d = nc.dram_tensor("w2_l", [128, KC_H, D], f32r, kind="ExternalInput").ap()
    bq_d = nc.dram_tensor("bq_l", [128, KC_D], f32, kind="ExternalInput").ap()
    bk_d = nc.dram_tensor("bk_l", [128, KC_D], f32, kind="ExternalInput").ap()
    b1_d = nc.dram_tensor("b1_l", [128, KC_H], f32, kind="ExternalInput").ap()
    mask_d = nc.dram_tensor("mask3", [T, 3 * T], f32, kind="ExternalInput").ap()
    id_d = nc.dram_tensor("ident", [128, 128], f32, kind="ExternalInput").ap()
    bv_d = bp_d = b2_d = None
    if use_bv:
        bv_d = nc.dram_tensor("bv_bc", [128, D], f32, kind="ExternalInput").ap()
    if use_bp:
        bp_d = nc.dram_tensor("bp_bc", [128, D], f32, kind="ExternalInput").ap()
    if use_b2:
        b2_d = nc.dram_tensor("b2_bc", [128, D], f32, kind="ExternalInput").ap()
    out_d = nc.dram_tensor("out", [BC, T, D], f32, kind="ExternalOutput").ap()

    with tile.TileContext(nc) as tc, ExitStack() as ctx:
        wpool = ctx.enter_context(tc.tile_pool(name="wpool", bufs=1))
        px = ctx.enter_context(tc.tile_pool(name="px", bufs=8))
        pxn = ctx.enter_context(tc.tile_pool(name="pxn", bufs=2))
        pst = ctx.enter_context(tc.tile_pool(name="pst", bufs=6))
        pxt = ctx.enter_context(tc.tile_pool(name="pxt", bufs=2))
        pqk = ctx.enter_context(tc.tile_pool(name="pqk", bufs=2))
        pv = ctx.enter_context(tc.tile_pool(name="pv", bufs=2))
        patt = ctx.enter_context(tc.tile_pool(name="patt", bufs=3))
        po = ctx.enter_context(tc.tile_pool(name="po", bufs=2))
        pot = ctx.enter_context(tc.tile_pool(name="pot", bufs=1))
        px2 = ctx.enter_context(tc.tile_pool(name="px2", bufs=1))
        phid = ctx.enter_context(tc.tile_pool(name="phid", bufs=1))
        pout = ctx.enter_context(tc.tile_pool(name="pout", bufs=3))
        # PSUM: 8 banks total -> (pool,tag) bufs must sum to <= 8
        psb = 2
        psq = ctx.enter_context(tc.tile_pool(name="psq", bufs=psb, space="PSUM"))
        pstk = ctx.enter_context(tc.tile_pool(name="pstk", bufs=psb, space="PSUM"))
        psa = ctx.enter_context(tc.tile_pool(name="psa", bufs=psb, space="PSUM"))
        psv = ctx.enter_context(tc.tile_pool(name="psv", bufs=psb, space="PSUM"))

        # resident weights / constants
        wq_sb = wpool.tile([128, KC_D, D], f32r)
        nc.sync.dma_start(out=wq_sb[:], in_=wq_d[:])
        wk_sb = wpool.tile([128, KC_D, D], f32r)
        nc.sync.dma_start(out=wk_sb[:], in_=wk_d[:])
        wv_sb = wpool.tile([128, KC_D, D], f32r)
        nc.sync.dma_start(out=wv_sb[:], in_=wv_d[:])
        wp_sb = wpool.tile([128, KC_D, D], f32r)
        nc.sync.dma_start(out=wp_sb[:], in_=wp_d[:])
        w1_sb = wpool.tile([128, KC_D, DH], f32r)
        nc.sync.dma_start(out=w1_sb[:], in_=w1_d[:])
        w2_sb = wpool.tile([128, KC_H, D], f32r)
        nc.sync.dma_start(out=w2_sb[:], in_=w2_d[:])
        bq_sb = wpool.tile([128, KC_D], f32)
        nc.sync.dma_start(out=bq_sb[:], in_=bq_d[:])
        bk_sb = wpool.tile([128, KC_D], f32)
        nc.sync.dma_start(out=bk_sb[:], in_=bk_d[:])
        b1_sb = wpool.tile([128, KC_H], f32)
        nc.sync.dma_start(out=b1_sb[:], in_=b1_d[:])
        mask_sb = wpool.tile([T, 3 * T], f32)
        nc.sync.dma_start(out=mask_sb[:], in_=mask_d[:])
        id_sb = wpool.tile([128, 128], f32)
        nc.sync.dma_start(out=id_sb[:], in_=id_d[:])
        eps_sb = wpool.tile([128, 1], f32)
        nc.vector.memset(eps_sb[:], EPS)
        bv_sb = bp_sb = b2_sb = None
        if use_bv:
            bv_sb = wpool.tile([128, D], f32)
            nc.sync.dma_start(out=bv_sb[:], in_=bv_d[:])
        if use_bp:
            bp_sb = wpool.tile([128, D], f32)
            nc.sync.dma_start(out=bp_sb[:], in_=bp_d[:])
        if use_b2:
            b2_sb = wpool.tile([128, D], f32)
            nc.sync.dma_start(out=b2_sb[:], in_=b2_d[:])

        assert D <= nc.vector.BN_STATS_FMAX

        def ln_normalize(x_sl, tp, tag):
            # x_sl: [tp, D] fp32 -> [tp, D] fp32 normalized (no affine)
            st = pst.tile([128, nc.vector.BN_STATS_DIM], f32, tag="bnst")
            nc.vector.bn_stats(out=st[:tp], in_=x_sl)
            mv = pst.tile([128, nc.vector.BN_AGGR_DIM], f32, tag="bnmv")
            nc.vector.bn_aggr(out=mv[:tp], in_=st[:tp])
            rs = pst.tile([128, 1], f32, tag="rs")
            nc.scalar.activation(out=rs[:tp], in_=mv[:tp, 1:2], func=AF.Sqrt,
                                 bias=eps_sb[:tp], scale=1.0)
            nc.vector.reciprocal(out=rs[:tp], in_=rs[:tp])
            nmu = pst.tile([128, 1], f32, tag="nmu")
            nc.vector.tensor_scalar(out=nmu[:tp], in0=mv[:tp, 0:1], scalar1=rs[:tp],
                                    scalar2=-1.0, op0=OP.mult, op1=OP.mult)
            xn = pxn.tile([128, D], f32, tag=tag)
            nc.scalar.activation(out=xn[:tp], in_=x_sl, func=AF.Identity,
                                 bias=nmu[:tp], scale=rs[:tp])
            return xn

        def ln_transpose(xn, dstT, tp, ioff):
            # xn [tp, D] -> dstT[:, c, ioff:ioff+tp] (feature-major, f32r)
            ps = psq.tile([128, KC_D, 128], f32, tag="ps_big")
            for c in range(KC_D):
                nc.tensor.transpose(ps[:128, c, :tp],
                                    xn[:tp, ts(c, 128)], id_sb[:tp, :tp])
            nc.vector.tensor_copy(dstT[:, :, ioff:ioff + tp], ps[:, :, :tp])

        xf = x_d.flatten_outer_dims()
        of = out_d.flatten_outer_dims()

        groups = []
        tok = 0
        nbat = BC
        while nbat > 0:
            nb = min(NB, nbat)
            groups.append((tok, nb))
            tok += nb * T
            nbat -= nb

        # per-group live tensors, carried between pipeline stages
        live = {}

        def emit_abc(gi):
            """LN1 + transpose + QKV projections for group gi.

            Generator: yields ~9 times so the caller can interleave these
            PE-dense chunks into group gi-1's gap-prone attention phase
            (keeps the tensor engine HAM-warm)."""
            tok0, nb = groups[gi]
            ntok = nb * T
            tp = ntok // 4
            XnT = pxt.tile([128, KC_D, NTOK_MAX], f32r, tag="xnt")
            xres = []
            for i in range(4):
                xt_ = px.tile([128, D], f32, tag="xres")
                nc.sync.dma_start(out=xt_[:tp], in_=xf[tok0 + i * tp: tok0 + (i + 1) * tp])
                xres.append(xt_)
                xn = ln_normalize(xt_[:tp], tp, "xn1")
                ln_transpose(xn, XnT, tp, i * tp)
                yield
            QT = pqk.tile([128, KC_D, NTOK_MAX], bf16, tag="qt")
            KT = pqk.tile([128, KC_D, NTOK_MAX], bf16, tag="kt")
            for di, (dst, w_sb, b_sb) in enumerate(((QT, wq_sb, bq_sb), (KT, wk_sb, bk_sb))):
                for m in range(KC_D):
                    ps = psq.tile([128, NTOK_MAX], f32, tag="ps_big")
                    for kc in range(KC_D):
                        nc.tensor.matmul(ps[:, :ntok], w_sb[:, kc, ts(m, 128)],
                                         XnT[:, kc, :ntok],
                                         start=(kc == 0), stop=(kc == KC_D - 1))
                    if di == 0:
                        nc.scalar.activation(out=dst[:, m, :ntok], in_=ps[:, :ntok],
                                             func=AF.Identity, bias=b_sb[:, m:m + 1],
                                             scale=1.0)
                    else:
                        nc.vector.tensor_scalar_add(out=dst[:, m, :ntok],
                                                    in0=ps[:, :ntok],
                                                    scalar1=b_sb[:, m:m + 1])
                yield
            V = pv.tile([128, NB, H, VW], bf16, tag="v")
            nc.vector.memset(V[:T, :nb, :, 64:VW], 1.0)
            for b in range(nb):
                ps = pstk.tile([128, D], f32, tag="ps_tok")
                for kc in range(KC_D):
                    nc.tensor.matmul(ps[:T, :], XnT[:, kc, b * T:(b + 1) * T],
                                     wv_sb[:, kc, :],
                                     start=(kc == 0), stop=(kc == KC_D - 1))
                psh = ps[:T].rearrange("p (h s) -> p h s", h=H)
                if use_bv:
                    bvh = bv_sb[:T].rearrange("p (h s) -> p h s", h=H)
                    nc.vector.tensor_add(V[:T, b, :, 0:64], psh, bvh)
                else:
                    nc.vector.tensor_copy(V[:T, b, :, 0:64], psh)
                if b % 2 == 1:
                    yield
            live[gi] = dict(xres=xres, QT=QT, KT=KT, V=V)
            yield

        def emit_attn(gi):
            """Attention for group gi -> feature-major OT. Yields per batch."""
            tok0, nb = groups[gi]
            QT, KT, V = live[gi]["QT"], live[gi]["KT"], live[gi]["V"]
            OT = pot.tile([128, KC_D, NTOK_MAX], f32r, tag="ot")
            for b in range(nb):
                bs = slice(b * T, (b + 1) * T)
                O_sb = po.tile([128, D], f32, tag="o")
                rden = pst.tile([128, H], f32, tag="rden")
                for g3 in range(2):
                    # same (h%2) within a triple: one PE row-group, so the
                    # three same-bank matmuls issue sequentially (concurrent
                    # same-bank PSUM writes are a hardware fault)
                    hs3 = [g3, g3 + 2, g3 + 4]
                    ps_s = psa.tile([128, 3, T], f32, tag="ps_att")
                    for j, h in enumerate(hs3):
                        mb, mo = h // 2, (h % 2) * 64
                        nc.tensor.matmul(ps_s[:T, j, :],
                                         KT[mo:mo + 64, mb, bs],
                                         QT[mo:mo + 64, mb, bs],
                                         start=True, stop=True)
                    sm = patt.tile([128, 3 * T], f32, tag="sm")
                    nc.vector.tensor_add(sm[:T], ps_s[:T, :, :], mask_sb[:T])
                    ex = patt.tile([128, 3 * T], bf16, tag="ex")
                    nc.scalar.activation(out=ex[:T], in_=sm[:T], func=AF.Exp)
                    ps_o = psv.tile([128, 3, VW], f32, tag="ps_av")
                    for j, h in enumerate(hs3):
                        nc.tensor.matmul(ps_o[:T, j, :],
                                         ex[:T, ts(j, T)],
                                         V[:T, b, h, :],
                                         start=True, stop=True)
                    nc.vector.reciprocal(out=rden[:T, g3 * 3:(g3 + 1) * 3],
                                         in_=ps_o[:T, :, 64:65])
                    for j, h in enumerate(hs3):
                        nc.vector.tensor_scalar_mul(
                            out=O_sb[:T, ts(h, 64)], in0=ps_o[:T, j, 0:64],
                            scalar1=rden[:T, g3 * 3 + j:g3 * 3 + j + 1])
                ps_t = psa.tile([128, KC_D, T], f32, tag="ps_att")
                for c in range(KC_D):
                    nc.tensor.transpose(ps_t[:128, c, :],
                                        O_sb[:T, ts(c, 128)], id_sb[:T, :T])
                nc.vector.tensor_copy(OT[:, :, bs], ps_t[:, :, :])
                yield
            live[gi]["OT"] = OT

        def emit_tail(gi):
            """proj+residual, LN2, FFN, store for group gi."""
            tok0, nb = groups[gi]
            ntok = nb * T
            tp = ntok // 4
            xres, OT = live[gi]["xres"], live[gi]["OT"]
            X2 = px2.tile([128, 4, D], f32, tag="x2")
            for i in range(4):
                ps = pstk.tile([128, D], f32, tag="ps_tok")
                for kc in range(KC_D):
                    nc.tensor.matmul(ps[:tp, :], OT[:, kc, i * tp:(i + 1) * tp],
                                     wp_sb[:, kc, :],
                                     start=(kc == 0), stop=(kc == KC_D - 1))
                if use_bp:
                    nc.vector.tensor_add(ps[:tp, :], ps[:tp, :], bp_sb[:tp, :])
                nc.vector.tensor_add(X2[:tp, i, :], ps[:tp, :], xres[i][:tp, :])
            Xn2T = pxt.tile([128, KC_D, NTOK_MAX], f32r, tag="xn2t")
            for i in range(4):
                xn2 = ln_normalize(X2[:tp, i, :], tp, "xn2")
                ln_transpose(xn2, Xn2T, tp, i * tp)
            HT = phid.tile([128, KC_H, NTOK_MAX], f32r, tag="hid")
            for m in range(KC_H):
                ps = psq.tile([128, NTOK_MAX], f32, tag="ps_big")
                for kc in range(KC_D):
                    nc.tensor.matmul(ps[:, :ntok], w1_sb[:, kc, ts(m, 128)],
                                     Xn2T[:, kc, :ntok],
                                     start=(kc == 0), stop=(kc == KC_D - 1))
                if m % 2 == 0:
                    nc.scalar.activation(out=HT[:, m, :ntok], in_=ps[:, :ntok],
                                         func=AF.Relu, bias=b1_sb[:, m:m + 1],
                                         scale=1.0)
                else:
                    nc.vector.tensor_scalar(out=HT[:, m, :ntok], in0=ps[:, :ntok],
                                            scalar1=b1_sb[:, m:m + 1], scalar2=0.0,
                                            op0=OP.add, op1=OP.max)
            for i in range(4):
                ps = pstk.tile([128, D], f32, tag="ps_tok")
                for kc in range(KC_H):
                    nc.tensor.matmul(ps[:tp, :], HT[:, kc, i * tp:(i + 1) * tp],
                                     w2_sb[:, kc, :],
                                     start=(kc == 0), stop=(kc == KC_H - 1))
                if use_b2:
                    nc.vector.tensor_add(ps[:tp, :], ps[:tp, :], b2_sb[:tp, :])
                ot_ = pout.tile([128, D], f32, tag="outt")
                nc.vector.tensor_add(ot_[:tp, :], ps[:tp, :], X2[:tp, i, :])
                nc.sync.dma_start(out=of[tok0 + i * tp: tok0 + (i + 1) * tp],
                                  in_=ot_[:tp, :])
            del live[gi]

        # software pipeline: attention(g-1) interleaved with prep(g)
        for g in range(len(groups) + 1):
            it_abc = emit_abc(g) if g < len(groups) else None
            it_d = emit_attn(g - 1) if g >= 1 else None
            while it_d is not None or it_abc is not None:
                if it_d is not None:
                    try:
                        next(it_d)
                    except StopIteration:
                        it_d = None
                if it_abc is not None:
                    for _ in range(2):
                        try:
                            next(it_abc)
                        except StopIteration:
                            it_abc = None
                            break
            if g >= 1:
                emit_tail(g - 1)

    nc.compile()
    return nc


def _get_nc(use_bv, use_bp, use_b2):
    key = (use_bv, use_bp, use_b2)
    if key not in _NC_CACHE:
        _NC_CACHE[key] = _build_nc(*key)
    return _NC_CACHE[key]


def _prep_inputs(x, wq, wk, wv, wproj, bproj, w1, b1, w2, b2, g1, beta1, g2, beta2):
    f = np.float32
    # stack per-head projections into [D, D] with head h at columns h*HS:(h+1)*HS
    wq_f = np.ascontiguousarray(wq.transpose(1, 0, 2).reshape(D, D), dtype=f)
    wk_f = np.ascontiguousarray(wk.transpose(1, 0, 2).reshape(D, D), dtype=f)
    wv_f = np.ascontiguousarray(wv.transpose(1, 0, 2).reshape(D, D), dtype=f)
    scale = np.float32(HS ** 0.5)
    # fold LN1 affine into qkv weights, LN2 affine into w1
    wq_p = (g1[:, None] * wq_f) * scale
    wk_p = g1[:, None] * wk_f
    wv_p = g1[:, None] * wv_f
    w1_p = g2[:, None] * w1
    bq = (beta1 @ wq_f) * scale
    bk = beta1 @ wk_f
    bv = beta1 @ wv_f
    b1_p = b1 + beta2 @ w1
    bp = bproj
    b2_p = b2

    def lay(w, kc):
        # [K, M] -> [128, kc, M] with K split into kc chunks of 128
        return np.ascontiguousarray(
            w.reshape(kc, 128, w.shape[1]).transpose(1, 0, 2), dtype=f)

    def layb(bias, kc):
        return np.ascontiguousarray(bias.reshape(kc, 128).T, dtype=f)

    # transposed causal mask, tiled for 3 heads: keep (t >= u)
    maskT = np.full((T, T), MASK_VAL, dtype=f)
    maskT[np.triu_indices(T)] = 0.0
    mask3 = np.ascontiguousarray(np.tile(maskT, (1, 3)))

    shared = {
        "wq_l": lay(wq_p, KC_D), "wk_l": lay(wk_p, KC_D), "wv_l": lay(wv_p, KC_D),
        "wp_l": lay(np.asarray(wproj, dtype=f), KC_D), "w1_l": lay(w1_p, KC_D),
        "w2_l": lay(np.asarray(w2, dtype=f), KC_H),
        "bq_l": layb(bq, KC_D), "bk_l": layb(bk, KC_D), "b1_l": layb(b1_p, KC_H),
        "mask3": mask3, "ident": np.eye(128, dtype=f),
    }
    use_bv = bool(np.any(bv))
    use_bp = bool(np.any(bp))
    use_b2 = bool(np.any(b2_p))
    if use_bv:
        shared["bv_bc"] = np.ascontiguousarray(np.tile(bv.astype(f), (128, 1)))
    if use_bp:
        shared["bp_bc"] = np.ascontiguousarray(np.tile(np.asarray(bp, f), (128, 1)))
    if use_b2:
        shared["b2_bc"] = np.ascontiguousarray(np.tile(np.asarray(b2_p, f), (128, 1)))
    return shared, (use_bv, use_bp, use_b2)


def kernel(**inputs):
    from concourse.bass_utils import run_bass_kernel_spmd

    x = np.asarray(inputs["x"], dtype=np.float32)
    shared, flags = _prep_inputs(
        x, *[np.asarray(inputs[k], dtype=np.float32) for k in
             ("wq", "wk", "wv", "wproj", "bproj", "w1", "b1", "w2", "b2",
              "g1", "beta1", "g2", "beta2")])
    nc = _get_nc(*flags)
    in_maps = []
    for c in range(N_CORES):
        m = dict(shared)
        m["x"] = np.ascontiguousarray(x[c * BC:(c + 1) * BC])
        in_maps.append(m)
    res = run_bass_kernel_spmd(nc, in_maps, core_ids=list(range(N_CORES)))
    return np.concatenate([res.results[i]["out"] for i in range(N_CORES)], axis=0)


# revision 18
# speedup vs baseline: 2.5907x; 1.0006x over previous
"""Trainium2 Bass kernel for a dense pre-norm transformer block.

Reference semantics (per batch b, seq len T=100, d_model D=384, heads H=6):
  h   = LN(x) * g1 + beta1
  q,k,v = per-head projections of h;  wei = softmax(mask(q k^T * sqrt(64)))
  x2  = x + (wei v) Wp + bp
  out = x2 + relu(LN(x2)*g2+beta2 @ W1 + b1) @ W2 + b2

Distribution: data-parallel over the batch dim across 8 NeuronCores
(64 batches each); weights are replicated. No collectives.

Host-side folds: LN affines are folded into the adjacent matmul
weights/biases; the sqrt(head_size) score scale is folded into Wq.
Big matmuls run as float32r (reduced-precision fp32 at full PE rate).

Attention avoids all per-head transposes: scores are computed
transposed ([u,t] = K^T-slice stationary x Q^T-slice moving), the
softmax denominator comes from ones-columns appended to V inside the
same AV matmul, and the 1/den scaling happens in token-major layout
where it is a per-partition scalar.
"""

import numpy as np
from contextlib import ExitStack

B, T, D = 512, 100, 384
H, HS = 6, 64
DH = 4 * D                      # FFN hidden 1536
N_CORES = 8
BC = B // N_CORES               # batches per core
EPS = 1e-5
MASK_VAL = -1e9
NB = 5                          # batches per inner group
NTOK_MAX = NB * T               # 500
KC_D = D // 128                 # 3 contraction chunks over D
KC_H = DH // 128                # 12 contraction chunks over DH
VW = 66                         # V columns per head: 64 + 2 ones (denominator)

_NC_CACHE = {}


def _build_nc(use_bv, use_bp, use_b2):
    import concourse.bass as bass
    import concourse.tile as tile
    from concourse import bacc, mybir

    f32 = mybir.dt.float32
    f32r = mybir.dt.float32r
    bf16 = mybir.dt.bfloat16
    AF = mybir.ActivationFunctionType
    OP = mybir.AluOpType
    ts = bass.ts

    nc = bacc.Bacc("TRN2", target_bir_lowering=False, debug=False,
                   enable_asserts=True, num_devices=N_CORES)

    x_d = nc.dram_tensor("x", [BC, T, D], f32, kind="ExternalInput").ap()
    wq_d = nc.dram_tensor("wq_l", [128, KC_D, D], f32r, kind="ExternalInput").ap()
    wk_d = nc.dram_tensor("wk_l", [128, KC_D, D], f32r, kind="ExternalInput").ap()
    wv_d = nc.dram_tensor("wv_l", [128, KC_D, D], f32r, kind="ExternalInput").ap()
    wp_d = nc.dram_tensor("wp_l", [128, KC_D, D], f32r, kind="ExternalInput").ap()
    w1_d = nc.dram_tensor("w1_l", [128, KC_D, DH], f32r, kind="ExternalInput").ap()
    w2_d = nc.dram_tensor("w2_l", [128, KC_H, D], f32r, kind="ExternalInput").ap()
    bq_d = nc.dram_tensor("bq_l", [128, KC_D], f32, kind="ExternalInput").ap()
    bk_d = nc.dram_tensor("bk_l", [128, KC_D], f32, kind="ExternalInput").ap()
    b1_d = nc.dram_tensor("b1_l", [128, KC_H], f32, kind="ExternalInput").ap()
    mask_d = nc.dram_tensor("mask3", [T, 3 * T], f32, kind="ExternalInput").ap()
    id_d = nc.dram_tensor("ident", [128, 128], f32, kind="ExternalInput").ap()
    bv_d = bp_d = b2_d = None
    if use_bv:
        bv_d = nc.dram_tensor("bv_bc", [128, D], f32, kind="ExternalInput").ap()
    if use_bp:
        bp_d = nc.dram_tensor("bp_bc", [128, D], f32, kind="ExternalInput").ap()
    if use_b2:
        b2_d = nc.dram_tensor("b2_bc", [128, D], f32, kind="ExternalInput").ap()
    out_d = nc.dram_tensor("out", [BC, T, D], f32, kind="ExternalOutput").ap()

    with tile.TileContext(nc) as tc, ExitStack() as ctx:
        wpool = ctx.enter_context(tc.tile_pool(name="wpool", bufs=1))
        px = ctx.enter_context(tc.tile_pool(name="px", bufs=8))
        pxn = ctx.enter_context(tc.tile_pool(name="pxn", bufs=2))
        pst = ctx.enter_context(tc.tile_pool(name="pst", bufs=6))
        pxt = ctx.enter_context(tc.tile_pool(name="pxt", bufs=2))
        pqk = ctx.enter_context(tc.tile_pool(name="pqk", bufs=2))
        pv = ctx.enter_context(tc.tile_pool(name="pv", bufs=2))
        patt = ctx.enter_context(tc.tile_pool(name="patt", bufs=3))
        po = ctx.enter_context(tc.tile_pool(name="po", bufs=2))
        pot = ctx.enter_context(tc.tile_pool(name="pot", bufs=1))
        px2 = ctx.enter_context(tc.tile_pool(name="px2", bufs=1))
        phid = ctx.enter_context(tc.tile_pool(name="phid", bufs=1))
        pout = ctx.enter_context(tc.tile_pool(name="pout", bufs=3))
        # PSUM: 8 banks total -> (pool,tag) bufs must sum to <= 8
        psb = 2
        psq = ctx.enter_context(tc.tile_pool(name="psq", bufs=psb, space="PSUM"))
        pstk = ctx.enter_context(tc.tile_pool(name="pstk", bufs=psb, space="PSUM"))
        psa = ctx.enter_context(tc.tile_pool(name="psa", bufs=psb, space="PSUM"))
        psv = ctx.enter_context(tc.tile_pool(name="psv", bufs=psb, space="PSUM"))

        # resident weights / constants
        wq_sb = wpool.tile([128, KC_D, D], f32r)
        nc.sync.dma_start(out=wq_sb[:], in_=wq_d[:])
        wk_sb = wpool.tile([128, KC_D, D], f32r)
        nc.sync.dma_start(out=wk_sb[:], in_=wk_d[:])
        wv_sb = wpool.tile([128, KC_D, D], f32r)
        nc.sync.dma_start(out=wv_sb[:], in_=wv_d[:])
        wp_sb = wpool.tile([128, KC_D, D], f32r)
        nc.sync.dma_start(out=wp_sb[:], in_=wp_d[:])
        w1_sb = wpool.tile([128, KC_D, DH], f32r)
        nc.sync.dma_start(out=w1_sb[:], in_=w1_d[:])
        w2_sb = wpool.tile([128, KC_H, D], f32r)
        nc.sync.dma_start(out=w2_sb[:], in_=w2_d[:])
        bq_sb = wpool.tile([128, KC_D], f32)
        nc.sync.dma_start(out=bq_sb[:], in_=bq_d[:])
        bk_sb = wpool.tile([128, KC_D], f32)
        nc.sync.dma_start(out=bk_sb[:], in_=bk_d[:])
        b1_sb = wpool.tile([128, KC_H], f32)
        nc.sync.dma_start(out=b1_sb[:], in_=b1_d[:])
        mask_sb = wpool.tile([T, 3 * T], f32)
        nc.sync.dma_start(out=mask_sb[:], in_=mask_d[:])
        id_sb = wpool.tile([128, 128], f32)
        nc.sync.dma_start(out=id_sb[:], in_=id_d[:])
        eps_sb = wpool.tile([128, 1], f32)
        nc.vector.memset(eps_sb[:], EPS)
        bv_sb = bp_sb = b2_sb = None
        if use_bv:
            bv_sb = wpool.tile([128, D], f32)
            nc.sync.dma_start(out=bv_sb[:], in_=bv_d[:])
        if use_bp:
            bp_sb = wpool.tile([128, D], f32)
            nc.sync.dma_start(out=bp_sb[:], in_=bp_d[:])
        if use_b2:
            b2_sb = wpool.tile([128, D], f32)
            nc.sync.dma_start(out=b2_sb[:], in_=b2_d[:])

        assert D <= nc.vector.BN_STATS_FMAX

        def ln_normalize(x_sl, tp, tag):
            # x_sl: [tp, D] fp32 -> [tp, D] fp32 normalized (no affine)
            st = pst.tile([128, nc.vector.BN_STATS_DIM], f32, tag="bnst")
            nc.vector.bn_stats(out=st[:tp], in_=x_sl)
            mv = pst.tile([128, nc.vector.BN_AGGR_DIM], f32, tag="bnmv")
            nc.vector.bn_aggr(out=mv[:tp], in_=st[:tp])
            rs = pst.tile([128, 1], f32, tag="rs")
            nc.scalar.activation(out=rs[:tp], in_=mv[:tp, 1:2], func=AF.Sqrt,
                                 bias=eps_sb[:tp], scale=1.0)
            nc.vector.reciprocal(out=rs[:tp], in_=rs[:tp])
            nmu = pst.tile([128, 1], f32, tag="nmu")
            nc.vector.tensor_scalar(out=nmu[:tp], in0=mv[:tp, 0:1], scalar1=rs[:tp],
                                    scalar2=-1.0, op0=OP.mult, op1=OP.mult)
            xn = pxn.tile([128, D], f32, tag=tag)
            nc.scalar.activation(out=xn[:tp], in_=x_sl, func=AF.Identity,
                                 bias=nmu[:tp], scale=rs[:tp])
            return xn

        def ln_transpose(xn, dstT, tp, ioff):
            # xn [tp, D] -> dstT[:, c, ioff:ioff+tp] (feature-major, f32r)
            ps = psq.tile([128, KC_D, 128], f32, tag="ps_big")
            for c in range(KC_D):
                nc.tensor.transpose(ps[:128, c, :tp],
                                    xn[:tp, ts(c, 128)], id_sb[:tp, :tp])
            nc.vector.tensor_copy(dstT[:, :, ioff:ioff + tp], ps[:, :, :tp])

        xf = x_d.flatten_outer_dims()
        of = out_d.flatten_outer_dims()

        groups = []
        tok = 0
        nbat = BC
        while nbat > 0:
            nb = min(NB, nbat)
            groups.append((tok, nb))
            tok += nb * T
            nbat -= nb

        # per-group live tensors, carried between pipeline stages
        live = {}

        def emit_abc(gi):
            """LN1 + transpose + QKV projections for group gi.

            Generator: yields ~9 times so the caller can interleave these
            PE-dense chunks into group gi-1's gap-prone attention phase
            (keeps the tensor engine HAM-warm)."""
            tok0, nb = groups[gi]
            ntok = nb * T
            tp = ntok // 4
            XnT = pxt.tile([128, KC_D, NTOK_MAX], f32r, tag="xnt")
            xres = []
            for i in range(4):
                xt_ = px.tile([128, D], f32, tag="xres")
                nc.sync.dma_start(out=xt_[:tp], in_=xf[tok0 + i * tp: tok0 + (i + 1) * tp])
                xres.append(xt_)
                xn = ln_normalize(xt_[:tp], tp, "xn1")
                ln_transpose(xn, XnT, tp, i * tp)
                yield
            QT = pqk.tile([128, KC_D, NTOK_MAX], bf16, tag="qt")
            KT = pqk.tile([128, KC_D, NTOK_MAX], bf16, tag="kt")
            for di, (dst, w_sb, b_sb) in enumerate(((QT, wq_sb, bq_sb), (KT, wk_sb, bk_sb))):
                for m in range(KC_D):
                    ps = psq.tile([128, NTOK_MAX], f32, tag="ps_big")
                    for kc in range(KC_D):
                        nc.tensor.matmul(ps[:, :ntok], w_sb[:, kc, ts(m, 128)],
                                         XnT[:, kc, :ntok],
                                         start=(kc == 0), stop=(kc == KC_D - 1))
                    if di == 0:
                        nc.scalar.activation(out=dst[:, m, :ntok], in_=ps[:, :ntok],
                                             func=AF.Identity, bias=b_sb[:, m:m + 1],
                                             scale=1.0)
                    else:
                        nc.vector.tensor_scalar_add(out=dst[:, m, :ntok],
                                                    in0=ps[:, :ntok],
                                                    scalar1=b_sb[:, m:m + 1])
                yield
            V = pv.tile([128, NB, H, VW], bf16, tag="v")
            nc.vector.memset(V[:T, :nb, :, 64:VW], 1.0)
            for b in range(nb):
                ps = pstk.tile([128, D], f32, tag="ps_tok")
                for kc in range(KC_D):
                    nc.tensor.matmul(ps[:T, :], XnT[:, kc, b * T:(b + 1) * T],
                                     wv_sb[:, kc, :],
                                     start=(kc == 0), stop=(kc == KC_D - 1))
                psh = ps[:T].rearrange("p (h s) -> p h s", h=H)
                if use_bv:
                    bvh = bv_sb[:T].rearrange("p (h s) -> p h s", h=H)
                    nc.vector.tensor_add(V[:T, b, :, 0:64], psh, bvh)
                else:
                    nc.vector.tensor_copy(V[:T, b, :, 0:64], psh)
                if b % 2 == 1:
                    yield
            live[gi] = dict(xres=xres, QT=QT, KT=KT, V=V)
            yield

        def emit_attn(gi):
            """Attention for group gi -> feature-major OT. Yields per batch."""
            tok0, nb = groups[gi]
            QT, KT, V = live[gi]["QT"], live[gi]["KT"], live[gi]["V"]
            OT = pot.tile([128, KC_D, NTOK_MAX], f32r, tag="ot")
            for b in range(nb):
                bs = slice(b * T, (b + 1) * T)
                O_sb = po.tile([128, D], f32, tag="o")
                rden = pst.tile([128, H], f32, tag="rden")
                for g3 in range(2):
                    # same (h%2) within a triple: one PE row-group, so the
                    # three same-bank matmuls issue sequentially (concurrent
                    # same-bank PSUM writes are a hardware fault)
                    hs3 = [g3, g3 + 2, g3 + 4]
                    ps_s = psa.tile([128, 3, T], f32, tag="ps_att")
                    for j, h in enumerate(hs3):
                        mb, mo = h // 2, (h % 2) * 64
                        nc.tensor.matmul(ps_s[:T, j, :],
                                         KT[mo:mo + 64, mb, bs],
                                         QT[mo:mo + 64, mb, bs],
                                         start=True, stop=True)
                    sm = patt.tile([128, 3 * T], f32, tag="sm")
                    nc.vector.tensor_add(sm[:T], ps_s[:T, :, :], mask_sb[:T])
                    ex = patt.tile([128, 3 * T], bf16, tag="ex")
                    nc.scalar.activation(out=ex[:T], in_=sm[:T], func=AF.Exp)
                    ps_o = psv.tile([128, 3, VW], f32, tag="ps_av")
                    for j, h in enumerate(hs3):
                        nc.tensor.matmul(ps_o[:T, j, :],
                                         ex[:T, ts(j, T)],
                                         V[:T, b, h, :],
                                         start=True, stop=True)
                    nc.vector.reciprocal(out=rden[:T, g3 * 3:(g3 + 1) * 3],
                                         in_=ps_o[:T, :, 64:65])
                    # one mul for the whole triple: rden broadcast along the
                    # 64-wide head slice via a stride-0 inner AP dim
                    rsl = rden[:T, g3 * 3:(g3 + 1) * 3]
                    rb = bass.AP(tensor=rsl.tensor, offset=rsl.offset,
                                 ap=[list(rsl.ap[0]), list(rsl.ap[1]), [0, 64]])
                    osl = O_sb[:T].rearrange("p (c two s) -> p c two s", two=2, s=64)[:, :, g3, :]
                    nc.vector.tensor_mul(osl, ps_o[:T, :, 0:64], rb)
                ps_t = psa.tile([128, KC_D, T], f32, tag="ps_att")
                for c in range(KC_D):
                    nc.tensor.transpose(ps_t[:128, c, :],
                                        O_sb[:T, ts(c, 128)], id_sb[:T, :T])
                nc.scalar.copy(out=OT[:, :, bs], in_=ps_t[:, :, :])
                yield
            live[gi]["OT"] = OT

        def emit_tail(gi):
            """proj+residual, LN2, FFN, store for group gi."""
            tok0, nb = groups[gi]
            ntok = nb * T
            tp = ntok // 4
            xres, OT = live[gi]["xres"], live[gi]["OT"]
            X2 = px2.tile([128, 4, D], f32, tag="x2")
            for i in range(4):
                ps = pstk.tile([128, D], f32, tag="ps_tok")
                for kc in range(KC_D):
                    nc.tensor.matmul(ps[:tp, :], OT[:, kc, i * tp:(i + 1) * tp],
                                     wp_sb[:, kc, :],
                                     start=(kc == 0), stop=(kc == KC_D - 1))
                if use_bp:
                    nc.vector.tensor_add(ps[:tp, :], ps[:tp, :], bp_sb[:tp, :])
                nc.vector.tensor_add(X2[:tp, i, :], ps[:tp, :], xres[i][:tp, :])
            Xn2T = pxt.tile([128, KC_D, NTOK_MAX], f32r, tag="xn2t")
            for i in range(4):
                xn2 = ln_normalize(X2[:tp, i, :], tp, "xn2")
                ln_transpose(xn2, Xn2T, tp, i * tp)
            HT = phid.tile([128, KC_H, NTOK_MAX], f32r, tag="hid")
            for m in range(KC_H):
                ps = psq.tile([128, NTOK_MAX], f32, tag="ps_big")
                for kc in range(KC_D):
                    nc.tensor.matmul(ps[:, :ntok], w1_sb[:, kc, ts(m, 128)],
                                     Xn2T[:, kc, :ntok],
                                     start=(kc == 0), stop=(kc == KC_D - 1))
                if m % 2 == 0:
                    nc.scalar.activation(out=HT[:, m, :ntok], in_=ps[:, :ntok],
                                         func=AF.Relu, bias=b1_sb[:, m:m + 1],
                                         scale=1.0)
                else:
                    nc.vector.tensor_scalar(out=HT[:, m, :ntok], in0=ps[:, :ntok],
                                            scalar1=b1_sb[:, m:m + 1], scalar2=0.0,
                                            op0=OP.add, op1=OP.max)
            for i in range(4):
                ps = pstk.tile([128, D], f32, tag="ps_tok")
                for kc in range(KC_H):
                    nc.tensor.matmul(ps[:tp, :], HT[:, kc, i * tp:(i + 1) * tp],
                                     w2_sb[:, kc, :],
                                     start=(kc == 0), stop=(kc == KC_H - 1))
                if use_b2:
                    nc.vector.tensor_add(ps[:tp, :], ps[:tp, :], b2_sb[:tp, :])
                ot_ = pout.tile([128, D], f32, tag="outt")
                nc.vector.tensor_add(ot_[:tp, :], ps[:tp, :], X2[:tp, i, :])
                nc.sync.dma_start(out=of[tok0 + i * tp: tok0 + (i + 1) * tp],
                                  in_=ot_[:tp, :])
            del live[gi]

        # software pipeline: attention(g-1) interleaved with prep(g)
        for g in range(len(groups) + 1):
            it_abc = emit_abc(g) if g < len(groups) else None
            it_d = emit_attn(g - 1) if g >= 1 else None
            while it_d is not None or it_abc is not None:
                if it_d is not None:
                    try:
                        next(it_d)
                    except StopIteration:
                        it_d = None
                if it_abc is not None:
                    for _ in range(2):
                        try:
                            next(it_abc)
                        except StopIteration:
                            it_abc = None
                            break
            if g >= 1:
                emit_tail(g - 1)

    nc.compile()
    return nc


def _get_nc(use_bv, use_bp, use_b2):
    key = (use_bv, use_bp, use_b2)
    if key not in _NC_CACHE:
        _NC_CACHE[key] = _build_nc(*key)
    return _NC_CACHE[key]


def _prep_inputs(x, wq, wk, wv, wproj, bproj, w1, b1, w2, b2, g1, beta1, g2, beta2):
    f = np.float32
    # stack per-head projections into [D, D] with head h at columns h*HS:(h+1)*HS
    wq_f = np.ascontiguousarray(wq.transpose(1, 0, 2).reshape(D, D), dtype=f)
    wk_f = np.ascontiguousarray(wk.transpose(1, 0, 2).reshape(D, D), dtype=f)
    wv_f = np.ascontiguousarray(wv.transpose(1, 0, 2).reshape(D, D), dtype=f)
    scale = np.float32(HS ** 0.5)
    # fold LN1 affine into qkv weights, LN2 affine into w1
    wq_p = (g1[:, None] * wq_f) * scale
    wk_p = g1[:, None] * wk_f
    wv_p = g1[:, None] * wv_f
    w1_p = g2[:, None] * w1
    bq = (beta1 @ wq_f) * scale
    bk = beta1 @ wk_f
    bv = beta1 @ wv_f
    b1_p = b1 + beta2 @ w1
    bp = bproj
    b2_p = b2

    def lay(w, kc):
        # [K, M] -> [128, kc, M] with K split into kc chunks of 128
        return np.ascontiguousarray(
            w.reshape(kc, 128, w.shape[1]).transpose(1, 0, 2), dtype=f)

    def layb(bias, kc):
        return np.ascontiguousarray(bias.reshape(kc, 128).T, dtype=f)

    # transposed causal mask, tiled for 3 heads: keep (t >= u)
    maskT = np.full((T, T), MASK_VAL, dtype=f)
    maskT[np.triu_indices(T)] = 0.0
    mask3 = np.ascontiguousarray(np.tile(maskT, (1, 3)))

    shared = {
        "wq_l": lay(wq_p, KC_D), "wk_l": lay(wk_p, KC_D), "wv_l": lay(wv_p, KC_D),
        "wp_l": lay(np.asarray(wproj, dtype=f), KC_D), "w1_l": lay(w1_p, KC_D),
        "w2_l": lay(np.asarray(w2, dtype=f), KC_H),
        "bq_l": layb(bq, KC_D), "bk_l": layb(bk, KC_D), "b1_l": layb(b1_p, KC_H),
        "mask3": mask3, "ident": np.eye(128, dtype=f),
    }
    use_bv = bool(np.any(bv))
    use_bp = bool(np.any(bp))
    use_b2 = bool(np.any(b2_p))
    if use_bv:
        shared["bv_bc"] = np.ascontiguousarray(np.tile(bv.astype(f), (128, 1)))
    if use_bp:
        shared["bp_bc"] = np.ascontiguousarray(np.tile(np.asarray(bp, f), (128, 1)))
    if use_b2:
        shared["b2_bc"] = np.ascontiguousarray(np.tile(np.asarray(b2_p, f), (128, 1)))
    return shared, (use_bv, use_bp, use_b2)


def kernel(**inputs):
    from concourse.bass_utils import run_bass_kernel_spmd

    x = np.asarray(inputs["x"], dtype=np.float32)
    shared, flags = _prep_inputs(
        x, *[np.asarray(inputs[k], dtype=np.float32) for k in
             ("wq", "wk", "wv", "wproj", "bproj", "w1", "b1", "w2", "b2",
              "g1", "beta1", "g2", "beta2")])
    nc = _get_nc(*flags)
    in_maps = []
    for c in range(N_CORES):
        m = dict(shared)
        m["x"] = np.ascontiguousarray(x[c * BC:(c + 1) * BC])
        in_maps.append(m)
    res = run_bass_kernel_spmd(nc, in_maps, core_ids=list(range(N_CORES)))
    return np.concatenate([res.results[i]["out"] for i in range(N_CORES)], axis=0)
